# revision 2
# baseline (speedup 1.0000x reference)
"""LIF bank Trainium2 Bass kernel, v6.

Per-lane recurrence over T=1000 steps, data-parallel over B*N lanes:
8 cores x 4096 lanes ([128 partitions, 32 free] tiles), time-major chunk
layout [P, T, F].

Structure (vs v4 baseline):
- A-form recurrence removes the post-reset V from the critical cycle:
    A_t   = fl(alpha*W_t + P_{t+1})         (STT)
    WR_t  = sel(D_t >= 0, Fw_t, 0)          (custom mask)
    W_t+1 = A_t - WR_t                      (TT sub)
    D_t+1 = W_t+1 - TH_t                    (TT sub)   [exact spike margin]
    TH_t  = (TH_t-1 * beta + c) + gamma*[D_t >= 0]     (custom affsel)
    P_t+2 = sel(D_t >= 0, 0, u_t+2)         (custom gate; 2-step refractory
                                             = two chained gates)
    Fw_t+1= fl(fl(alpha*TH_t) + P_{t+2})    (STT)
  Cycle = D->MASK->SUBW->DTT (3 hops) instead of W->S->M->W with V inline.
- Device outputs are the raw W and D histories; the host derives
    s = (D >= 0),  v = where(D >= 0, D, W)
  which are exactly the device's own spike decisions / soft-reset values.
- Time-major chunk layout (inner stride 1) avoids strided-write stalls.

Numerics vs the jax reference: everything fp32-exact except the spiking
branch of W (fl(aW+P)-fl(fl(aTH)+P) vs fl(a*fl(W-TH)), ~2ulp), same class
of deviation the v4 baseline already had via its c_imm constant.
"""

import numpy as np

ALPHA = 0.95
BETA = 0.995   # THETA_DECAY
GAMMA = 0.35   # THETA_INC

B, N, T = 16, 2048, 1000
NCORES = 8
NSH = N // NCORES          # 256 neurons per core
P, F = 128, 32             # lanes per core = P*F = B*NSH = 4096
TC = 50                    # timesteps per chunk

_CACHE = {}


def _register_custom_ops():
    import concourse.dve_ops as dvo
    from concourse.dve_spec import (
        Spec, Src0, Src1, C0, C1, C2, Zero, One, select, lower, _has_src1,
        PageIdx,
    )
    from concourse.dve_uop import DveOpSpec

    if "LIF2_MASKGATE" in dvo._SUB_OPCODE_FOR_NAME:
        return {o.name: o for o in dvo.OPS if o.name.startswith("LIF2_")}

    def _mg_ref(in0, in1, s0, s1, imm2):
        pg = np.arange(in0.shape[1], dtype=np.float32).reshape(1, -1, 1)
        return np.where(in0 >= 0, in1 * (1.0 - pg), in1 * pg).astype(np.float32)

    _pg = PageIdx(Zero, One)
    specs = {
        # page 0: sel(D>=0, Src1, 0) = WR;  page 1: sel(D>=0, 0, Src1) = P
        "LIF2_MASKGATE": Spec(
            body=select(Src0 >= Zero, Src1 * (One - _pg), Src1 * _pg),
            reference=_mg_ref,
        ),
        # WR = sel(D >= 0, Fw, 0)
        "LIF2_MASK": Spec(
            body=select(Src0 >= Zero, Src1, Zero),
            reference=lambda in0, in1, s0, s1, imm2: np.where(
                in0 >= 0, in1, np.float32(0.0)).astype(np.float32),
        ),
        # P = sel(D >= 0, 0, u)
        "LIF2_GATE": Spec(
            body=select(Src0 >= Zero, Zero, Src1),
            reference=lambda in0, in1, s0, s1, imm2: np.where(
                in0 >= 0, np.float32(0.0), in1).astype(np.float32),
        ),
        # TH' = (TH*C0 + C1) + (D >= 0)*C2
        "LIF2_AFFSEL": Spec(
            body=(Src0 * C0 + C1) + (Src1 >= Zero) * C2,
            reference=lambda in0, in1, s0, s1, imm2: (
                (in0 * np.float32(s0) + np.float32(s1))
                + (in1 >= 0).astype(np.float32) * np.float32(imm2)
            ).astype(np.float32),
        ),
    }
    new_ops = []
    base = len(dvo.OPS)
    for i, (name, spec) in enumerate(specs.items()):
        opcode = dvo._CUSTOM_DVE_ROW_BASE + base + i
        shas = {}
        for ver in ("v3", "v4"):
            uops = lower(spec, ver=ver)
            shas[ver] = DveOpSpec(
                name=name, opcode=opcode, uops=uops, rd1_en=_has_src1(spec)
            ).sha(ver)
        dvo._SUB_OPCODE_FOR_NAME[name] = opcode
        new_ops.append(dvo.DveOp(name, spec, subdim=(name == "LIF2_MASKGATE"),
                                 uops_sha=shas))
    dvo.OPS.extend(new_ops)
    dvo.CUSTOM_DVE_SPECS.update({o.name: o.spec for o in new_ops})
    return {o.name: o for o in new_ops}


def _build_nc(t_total, tc, tb_val):
    import concourse.bacc as bacc
    import concourse.mybir as mybir
    import concourse.tile as tile

    ops = _register_custom_ops()
    MK, GT, AS = ops["LIF2_MASK"], ops["LIF2_GATE"], ops["LIF2_AFFSEL"]
    MG = ops["LIF2_MASKGATE"]

    f32 = mybir.dt.float32
    op = mybir.AluOpType

    tb = np.float32(tb_val)
    c_imm = float(np.float32(tb * np.float32(1.0 - BETA)))
    atb = float(np.float32(np.float32(ALPHA) * tb))  # fl(alpha*tb)

    nc = bacc.Bacc("TRN2", target_bir_lowering=False, num_devices=NCORES)
    u_d = nc.dram_tensor("u", [P, t_total, F], f32, kind="ExternalInput")
    s_d = nc.dram_tensor("s", [P, t_total, F], f32, kind="ExternalOutput")  # D history
    v_d = nc.dram_tensor("v", [P, t_total, F], f32, kind="ExternalOutput")  # W history

    nchunks = t_total // tc
    assert nchunks * tc == t_total
    vec = nc.vector

    with tile.TileContext(nc) as tc_ctx:
        with (
            tc_ctx.tile_pool(name="state", bufs=1) as st,
            tc_ctx.tile_pool(name="uc", bufs=3) as upool,
            tc_ctx.tile_pool(name="wd", bufs=2) as wpool,
        ):
            th = [st.tile([P, F], f32, tag=f"th{i}", name=f"th{i}") for i in range(2)]
            aa = [st.tile([P, F], f32, tag=f"aa{i}", name=f"aa{i}") for i in range(2)]
            wp = [st.tile([P, 2, F], f32, tag=f"wp{i}", name=f"wp{i}") for i in range(2)]

            uc, wd = {}, {}
            UH = 8  # head slots of chunk 0 loaded separately for fast start

            def load_u(k):
                if k < nchunks and k not in uc:
                    uc[k] = upool.tile([P, tc + 2, F], f32, tag="uc", name=f"uc{k}")
                    nc.sync.dma_start(uc[k][:, 2:, :], u_d[:, k * tc:(k + 1) * tc, :])

            uh = st.tile([P, UH, F], f32, tag="uh", name="uh")
            nc.sync.dma_start(uh[:], u_d[:, 0:UH, :])

            def u_at(g):
                g = min(g, t_total - 1)
                if g < UH:
                    return uh[:, g, :]
                return uc[g // tc][:, 2 + g % tc, :]

            load_u(0)
            load_u(1)

            wd[0] = wpool.tile([P, 2, tc + 1, F], f32, tag="wd", name="wd0")

            # ---- prologue: W_0 = u_0, D_0 = W_0 - tb, TH_-1 = tb,
            #      Fw_0 = fl(atb + u_1), A_0 = fl(alpha*W_0 + u_1)
            vec.tensor_copy(wd[0][:, 0, 0, :], uh[:, 0, :])
            vec.tensor_scalar(out=wd[0][:, 1, 0, :], in0=wd[0][:, 0, 0, :],
                              scalar1=float(tb), scalar2=None, op0=op.subtract)
            vec.memset(th[0][:], float(tb))
            vec.tensor_scalar(out=uc[0][:, 0, :], in0=uh[:, 1, :],
                              scalar1=atb, scalar2=None, op0=op.add)
            vec.scalar_tensor_tensor(out=aa[0][:], in0=wd[0][:, 0, 0, :],
                                     scalar=ALPHA, in1=uh[:, 1, :],
                                     op0=op.mult, op1=op.add)

            for t in range(t_total - 1):
                k, tau = t // tc, t % tc
                q, q2 = t % 2, (t + 1) % 2
                if tau == 0:
                    if k > 0:
                        wd[k] = wpool.tile([P, 2, tc + 1, F], f32, tag="wd",
                                           name=f"wd{k}")
                    load_u(k + 2)

                # chunk k's slot 0 lives in wd[k-1] slot tc (no carry copy)
                if tau == 0 and k > 0:
                    d_t = wd[k - 1][:, 1, tc, :]
                else:
                    d_t = wd[k][:, 1, tau, :]
                g = min(t + 2, t_total - 1)
                kg, tg = g // tc, 2 + g % tc
                if kg == k and g >= UH:
                    # 1+3 merged: [WR_t | P_{t+2}] via paged select
                    #   Src1 pages = [fw-slot q | u-slot tg] of uc[k]
                    step = tg - q
                    vec._custom_dve(
                        MG, out=wp[q][:],
                        in0=d_t.unsqueeze(1).broadcast_to([P, 2, F]),
                        in1=uc[k][:, q:tg + 1:step, :])
                else:
                    vec._custom_dve(MK, out=wp[q][:, 0, :], in0=d_t,
                                    in1=uc[k][:, q, :])
                    vec._custom_dve(GT, out=wp[q][:, 1, :], in0=d_t, in1=u_at(t + 2))
                # 2. TH_t = (TH_{t-1}*beta + c) + gamma*[D_t >= 0]
                vec._custom_dve(AS, out=th[q2][:], in0=th[q][:], in1=d_t,
                                s0=BETA, s1=c_imm, imm2=GAMMA)
                # 4. W_{t+1} = A_t - WR_t
                vec.tensor_tensor(out=wd[k][:, 0, tau + 1, :], in0=aa[q][:],
                                  in1=wp[q][:, 0, :], op=op.subtract)
                # 5. Fw_{t+1} = fl(fl(alpha*TH_t) + P_{t+2}) -> fw slot of chunk(t+1)
                k1 = (t + 1) // tc
                vec.scalar_tensor_tensor(out=uc[k1][:, q2, :], in0=th[q2][:],
                                         scalar=ALPHA, in1=wp[q][:, 1, :],
                                         op0=op.mult, op1=op.add)
                # 6. D_{t+1} = W_{t+1} - TH_t
                vec.tensor_tensor(out=wd[k][:, 1, tau + 1, :],
                                  in0=wd[k][:, 0, tau + 1, :], in1=th[q2][:],
                                  op=op.subtract)
                # 7. A_{t+1} = fl(alpha*W_{t+1} + P_{t+2})
                vec.scalar_tensor_tensor(out=aa[q2][:], in0=wd[k][:, 0, tau + 1, :],
                                         scalar=ALPHA, in1=wp[q][:, 1, :],
                                         op0=op.mult, op1=op.add)

                # stream the chunk's finished slots out in pieces so the
                # final chunk's DMA overlaps compute (slot tau+1 is written
                # by iter tau; slots [lo, tau+1) are final after iter tau)
                last = (k == nchunks - 1)
                cuts = (14, 26, 36, 44, tc - 1) if last else (tc - 8 - 1, tc - 1)
                if tau in cuts or t == t_total - 2:
                    lo = 0 if k == 0 else 1
                    prev_cut = [c for c in cuts if c < tau]
                    if prev_cut:
                        lo = prev_cut[-1] + 2
                    hi = min(tau + 2, t_total - k * tc)
                    if tau == tc - 1 or t == t_total - 2:
                        hi = min(tc + 1, t_total - k * tc)
                    if hi > lo:
                        nc.sync.dma_start(v_d[:, k * tc + lo:k * tc + hi, :],
                                          wd[k][:, 0, lo:hi, :])
                        nc.sync.dma_start(s_d[:, k * tc + lo:k * tc + hi, :],
                                          wd[k][:, 1, lo:hi, :])

    nc.compile()
    return nc


def _get_nc(t_total, tc, tb_val):
    key = (t_total, tc, float(tb_val))
    if key not in _CACHE:
        _CACHE[key] = _build_nc(t_total, tc, tb_val)
    return _CACHE[key]


def _shard_inputs(u, t_total):
    u = np.asarray(u, dtype=np.float32)
    in_maps = []
    for c in range(NCORES):
        lo, hi = c * NSH, (c + 1) * NSH
        # [B, NSH, T] -> [B, NSH//F, F, T] -> [B, NSH//F, T, F] -> [P, T, F]
        uc = np.ascontiguousarray(
            u[:, lo:hi, :t_total]
            .reshape(B, NSH // F, F, t_total)
            .transpose(0, 1, 3, 2)
            .reshape(P, t_total, F)
        )
        in_maps.append({"u": uc})
    return in_maps


def _unshard(res, t_total):
    s_full = np.empty((B, N, t_total), dtype=np.float32)
    v_full = np.empty((B, N, t_total), dtype=np.float32)
    for c in range(NCORES):
        lo, hi = c * NSH, (c + 1) * NSH
        d = res[c]["s"]  # [P, T, F] = D history
        w = res[c]["v"]  # [P, T, F] = W history
        spk = (d >= 0)
        s = spk.astype(np.float32)
        v = np.where(spk, d, w)
        for name, arr in (("s", s), ("v", v)):
            full = (
                arr.reshape(B, NSH // F, t_total, F)
                .transpose(0, 1, 3, 2)
                .reshape(B, NSH, t_total)
            )
            if name == "s":
                s_full[:, lo:hi, :] = full
            else:
                v_full[:, lo:hi, :] = full
    return s_full, v_full


def _host_fallback(u, theta_base):
    """Exact numpy simulation; only used if theta_base is non-uniform."""
    u = np.asarray(u, np.float32)
    b, n, t = u.shape
    tb = np.asarray(theta_base, np.float32)[0, :, 0]
    v = np.zeros((b, n), np.float32)
    theta = np.broadcast_to(tb, (b, n)).astype(np.float32).copy()
    ref = np.zeros((b, n), np.float32)
    c = (tb * np.float32(1.0 - BETA)).astype(np.float32)
    ss = np.empty((b, n, t), np.float32)
    vs = np.empty((b, n, t), np.float32)
    for i in range(t):
        u_eff = np.where(ref > 0, np.float32(0.0), u[:, :, i])
        v = (np.float32(ALPHA) * v + u_eff).astype(np.float32)
        s = (v >= theta).astype(np.float32)
        v = (v - s * theta).astype(np.float32)
        ref = np.where(s > 0, np.float32(2.0), np.maximum(ref - 1.0, 0.0).astype(np.float32))
        theta = ((theta * np.float32(BETA) + c) + np.float32(GAMMA) * s).astype(np.float32)
        ss[:, :, i] = s
        vs[:, :, i] = v
    return ss, vs


def run(u, theta_base, t_total=T, tc=TC, trace=False):
    from concourse.bass_utils import run_bass_kernel_spmd

    tb = np.asarray(theta_base, dtype=np.float32)
    nc = _get_nc(t_total, tc, float(tb.flat[0]))
    in_maps = _shard_inputs(u, t_total)
    res = run_bass_kernel_spmd(nc, in_maps, core_ids=list(range(NCORES)), trace=trace)
    s_full, v_full = _unshard(res.results, t_total)
    return (s_full, v_full), res


def kernel(u, theta_base):
    tb = np.asarray(theta_base, dtype=np.float32)
    if not np.all(tb == tb.flat[0]):
        return _host_fallback(u, theta_base)
    (s_full, v_full), _ = run(u, theta_base)
    return s_full, v_full


# revision 3
# speedup vs baseline: 1.0010x; 1.0010x over previous
"""LIF bank Trainium2 Bass kernel, v6.

Per-lane recurrence over T=1000 steps, data-parallel over B*N lanes:
8 cores x 4096 lanes ([128 partitions, 32 free] tiles), time-major chunk
layout [P, T, F].

Structure (vs v4 baseline):
- A-form recurrence removes the post-reset V from the critical cycle:
    A_t   = fl(alpha*W_t + P_{t+1})         (STT)
    WR_t  = sel(D_t >= 0, Fw_t, 0)          (custom mask)
    W_t+1 = A_t - WR_t                      (TT sub)
    D_t+1 = W_t+1 - TH_t                    (TT sub)   [exact spike margin]
    TH_t  = (TH_t-1 * beta + c) + gamma*[D_t >= 0]     (custom affsel)
    P_t+2 = sel(D_t >= 0, 0, u_t+2)         (custom gate; 2-step refractory
                                             = two chained gates)
    Fw_t+1= fl(fl(alpha*TH_t) + P_{t+2})    (STT)
  Cycle = D->MASK->SUBW->DTT (3 hops) instead of W->S->M->W with V inline.
- Device outputs are the raw W and D histories; the host derives
    s = (D >= 0),  v = where(D >= 0, D, W)
  which are exactly the device's own spike decisions / soft-reset values.
- Time-major chunk layout (inner stride 1) avoids strided-write stalls.

Numerics vs the jax reference: everything fp32-exact except the spiking
branch of W (fl(aW+P)-fl(fl(aTH)+P) vs fl(a*fl(W-TH)), ~2ulp), same class
of deviation the v4 baseline already had via its c_imm constant.
"""

import numpy as np

ALPHA = 0.95
BETA = 0.995   # THETA_DECAY
GAMMA = 0.35   # THETA_INC

B, N, T = 16, 2048, 1000
NCORES = 8
NSH = N // NCORES          # 256 neurons per core
P, F = 128, 32             # lanes per core = P*F = B*NSH = 4096
TC = 50                    # timesteps per chunk

_CACHE = {}


def _register_custom_ops():
    import concourse.dve_ops as dvo
    from concourse.dve_spec import (
        Spec, Src0, Src1, C0, C1, C2, Zero, One, select, lower, _has_src1,
        PageIdx,
    )
    from concourse.dve_uop import DveOpSpec

    if "LIF2_MASKGATE" in dvo._SUB_OPCODE_FOR_NAME:
        return {o.name: o for o in dvo.OPS if o.name.startswith("LIF2_")}

    def _mg_ref(in0, in1, s0, s1, imm2):
        pg = np.arange(in0.shape[1], dtype=np.float32).reshape(1, -1, 1)
        return np.where(in0 >= 0, in1 * (1.0 - pg), in1 * pg).astype(np.float32)

    _pg = PageIdx(Zero, One)
    specs = {
        # page 0: sel(D>=0, Src1, 0) = WR;  page 1: sel(D>=0, 0, Src1) = P
        "LIF2_MASKGATE": Spec(
            body=select(Src0 >= Zero, Src1 * (One - _pg), Src1 * _pg),
            reference=_mg_ref,
        ),
        # WR = sel(D >= 0, Fw, 0)
        "LIF2_MASK": Spec(
            body=select(Src0 >= Zero, Src1, Zero),
            reference=lambda in0, in1, s0, s1, imm2: np.where(
                in0 >= 0, in1, np.float32(0.0)).astype(np.float32),
        ),
        # P = sel(D >= 0, 0, u)
        "LIF2_GATE": Spec(
            body=select(Src0 >= Zero, Zero, Src1),
            reference=lambda in0, in1, s0, s1, imm2: np.where(
                in0 >= 0, np.float32(0.0), in1).astype(np.float32),
        ),
        # TH' = (TH*C0 + C1) + (D >= 0)*C2
        "LIF2_AFFSEL": Spec(
            body=(Src0 * C0 + C1) + (Src1 >= Zero) * C2,
            reference=lambda in0, in1, s0, s1, imm2: (
                (in0 * np.float32(s0) + np.float32(s1))
                + (in1 >= 0).astype(np.float32) * np.float32(imm2)
            ).astype(np.float32),
        ),
    }
    new_ops = []
    base = len(dvo.OPS)
    for i, (name, spec) in enumerate(specs.items()):
        opcode = dvo._CUSTOM_DVE_ROW_BASE + base + i
        shas = {}
        for ver in ("v3", "v4"):
            uops = lower(spec, ver=ver)
            shas[ver] = DveOpSpec(
                name=name, opcode=opcode, uops=uops, rd1_en=_has_src1(spec)
            ).sha(ver)
        dvo._SUB_OPCODE_FOR_NAME[name] = opcode
        new_ops.append(dvo.DveOp(name, spec, subdim=(name == "LIF2_MASKGATE"),
                                 uops_sha=shas))
    dvo.OPS.extend(new_ops)
    dvo.CUSTOM_DVE_SPECS.update({o.name: o.spec for o in new_ops})
    return {o.name: o for o in new_ops}


def _build_nc(t_total, tc, tb_val):
    import concourse.bacc as bacc
    import concourse.mybir as mybir
    import concourse.tile as tile

    ops = _register_custom_ops()
    MK, GT, AS = ops["LIF2_MASK"], ops["LIF2_GATE"], ops["LIF2_AFFSEL"]
    MG = ops["LIF2_MASKGATE"]

    f32 = mybir.dt.float32
    op = mybir.AluOpType

    tb = np.float32(tb_val)
    c_imm = float(np.float32(tb * np.float32(1.0 - BETA)))
    atb = float(np.float32(np.float32(ALPHA) * tb))  # fl(alpha*tb)

    nc = bacc.Bacc("TRN2", target_bir_lowering=False, num_devices=NCORES)
    u_d = nc.dram_tensor("u", [P, t_total, F], f32, kind="ExternalInput")
    s_d = nc.dram_tensor("s", [P, t_total, F], f32, kind="ExternalOutput")  # D history
    v_d = nc.dram_tensor("v", [P, t_total, F], f32, kind="ExternalOutput")  # W history

    nchunks = t_total // tc
    assert nchunks * tc == t_total
    vec = nc.vector

    with tile.TileContext(nc) as tc_ctx:
        with (
            tc_ctx.tile_pool(name="state", bufs=1) as st,
            tc_ctx.tile_pool(name="uc", bufs=3) as upool,
            tc_ctx.tile_pool(name="wd", bufs=2) as wpool,
        ):
            th = [st.tile([P, F], f32, tag=f"th{i}", name=f"th{i}") for i in range(2)]
            aa = [st.tile([P, F], f32, tag=f"aa{i}", name=f"aa{i}") for i in range(2)]
            wp = [st.tile([P, 2, F], f32, tag=f"wp{i}", name=f"wp{i}") for i in range(2)]

            uc, wd = {}, {}
            UH = 8  # head slots of chunk 0 loaded separately for fast start

            def load_u(k):
                if k < nchunks and k not in uc:
                    uc[k] = upool.tile([P, tc + 4, F], f32, tag="uc", name=f"uc{k}")
                    hi = min((k + 1) * tc + 2, t_total)
                    nc.sync.dma_start(uc[k][:, 2:2 + hi - k * tc, :],
                                      u_d[:, k * tc:hi, :])

            uh = st.tile([P, UH, F], f32, tag="uh", name="uh")
            nc.sync.dma_start(uh[:], u_d[:, 0:UH, :])

            def u_at(k, g):
                g = min(g, t_total - 1)
                if g < UH:
                    return uh[:, g, :]
                return uc[k][:, 2 + g - k * tc, :]

            load_u(0)
            load_u(1)

            wd[0] = wpool.tile([P, 2, tc + 1, F], f32, tag="wd", name="wd0")

            # ---- prologue: W_0 = u_0, D_0 = W_0 - tb, TH_-1 = tb,
            #      Fw_0 = fl(atb + u_1), A_0 = fl(alpha*W_0 + u_1)
            vec.tensor_copy(wd[0][:, 0, 0, :], uh[:, 0, :])
            vec.tensor_scalar(out=wd[0][:, 1, 0, :], in0=wd[0][:, 0, 0, :],
                              scalar1=float(tb), scalar2=None, op0=op.subtract)
            vec.memset(th[0][:], float(tb))
            vec.tensor_scalar(out=uc[0][:, 0, :], in0=uh[:, 1, :],
                              scalar1=atb, scalar2=None, op0=op.add)
            vec.scalar_tensor_tensor(out=aa[0][:], in0=wd[0][:, 0, 0, :],
                                     scalar=ALPHA, in1=uh[:, 1, :],
                                     op0=op.mult, op1=op.add)

            for t in range(t_total - 1):
                k, tau = t // tc, t % tc
                q, q2 = t % 2, (t + 1) % 2
                if tau == 0:
                    if k > 0:
                        wd[k] = wpool.tile([P, 2, tc + 1, F], f32, tag="wd",
                                           name=f"wd{k}")
                    load_u(k + 2)

                # chunk k's slot 0 lives in wd[k-1] slot tc (no carry copy)
                if tau == 0 and k > 0:
                    d_t = wd[k - 1][:, 1, tc, :]
                else:
                    d_t = wd[k][:, 1, tau, :]
                g = min(t + 2, t_total - 1)
                tg = 2 + g - k * tc
                if g >= UH:
                    # 1+3 merged: [WR_t | P_{t+2}] via paged select
                    #   Src1 pages = [fw-slot q | u-slot tg] of uc[k]
                    step = tg - q
                    vec._custom_dve(
                        MG, out=wp[q][:],
                        in0=d_t.unsqueeze(1).broadcast_to([P, 2, F]),
                        in1=uc[k][:, q:tg + 1:step, :])
                else:
                    vec._custom_dve(MK, out=wp[q][:, 0, :], in0=d_t,
                                    in1=uc[k][:, q, :])
                    vec._custom_dve(GT, out=wp[q][:, 1, :], in0=d_t,
                                    in1=u_at(k, t + 2))
                # 2. TH_t = (TH_{t-1}*beta + c) + gamma*[D_t >= 0]
                vec._custom_dve(AS, out=th[q2][:], in0=th[q][:], in1=d_t,
                                s0=BETA, s1=c_imm, imm2=GAMMA)
                # 4. W_{t+1} = A_t - WR_t
                vec.tensor_tensor(out=wd[k][:, 0, tau + 1, :], in0=aa[q][:],
                                  in1=wp[q][:, 0, :], op=op.subtract)
                # 5. Fw_{t+1} = fl(fl(alpha*TH_t) + P_{t+2}) -> fw slot of chunk(t+1)
                k1 = (t + 1) // tc
                vec.scalar_tensor_tensor(out=uc[k1][:, q2, :], in0=th[q2][:],
                                         scalar=ALPHA, in1=wp[q][:, 1, :],
                                         op0=op.mult, op1=op.add)
                # 6. D_{t+1} = W_{t+1} - TH_t
                vec.tensor_tensor(out=wd[k][:, 1, tau + 1, :],
                                  in0=wd[k][:, 0, tau + 1, :], in1=th[q2][:],
                                  op=op.subtract)
                # 7. A_{t+1} = fl(alpha*W_{t+1} + P_{t+2})
                vec.scalar_tensor_tensor(out=aa[q2][:], in0=wd[k][:, 0, tau + 1, :],
                                         scalar=ALPHA, in1=wp[q][:, 1, :],
                                         op0=op.mult, op1=op.add)

                # stream the chunk's finished slots out in pieces so the
                # final chunk's DMA overlaps compute (slot tau+1 is written
                # by iter tau; slots [lo, tau+1) are final after iter tau)
                last = (k == nchunks - 1)
                cuts = (14, 26, 36, 44, tc - 1) if last else (tc - 8 - 1, tc - 1)
                if tau in cuts or t == t_total - 2:
                    lo = 0 if k == 0 else 1
                    prev_cut = [c for c in cuts if c < tau]
                    if prev_cut:
                        lo = prev_cut[-1] + 2
                    hi = min(tau + 2, t_total - k * tc)
                    if tau == tc - 1 or t == t_total - 2:
                        hi = min(tc + 1, t_total - k * tc)
                    if hi > lo:
                        nc.sync.dma_start(v_d[:, k * tc + lo:k * tc + hi, :],
                                          wd[k][:, 0, lo:hi, :])
                        nc.sync.dma_start(s_d[:, k * tc + lo:k * tc + hi, :],
                                          wd[k][:, 1, lo:hi, :])

    nc.compile()
    return nc


def _get_nc(t_total, tc, tb_val):
    key = (t_total, tc, float(tb_val))
    if key not in _CACHE:
        _CACHE[key] = _build_nc(t_total, tc, tb_val)
    return _CACHE[key]


def _shard_inputs(u, t_total):
    u = np.asarray(u, dtype=np.float32)
    in_maps = []
    for c in range(NCORES):
        lo, hi = c * NSH, (c + 1) * NSH
        # [B, NSH, T] -> [B, NSH//F, F, T] -> [B, NSH//F, T, F] -> [P, T, F]
        uc = np.ascontiguousarray(
            u[:, lo:hi, :t_total]
            .reshape(B, NSH // F, F, t_total)
            .transpose(0, 1, 3, 2)
            .reshape(P, t_total, F)
        )
        in_maps.append({"u": uc})
    return in_maps


def _unshard(res, t_total):
    s_full = np.empty((B, N, t_total), dtype=np.float32)
    v_full = np.empty((B, N, t_total), dtype=np.float32)
    for c in range(NCORES):
        lo, hi = c * NSH, (c + 1) * NSH
        d = res[c]["s"]  # [P, T, F] = D history
        w = res[c]["v"]  # [P, T, F] = W history
        spk = (d >= 0)
        s = spk.astype(np.float32)
        v = np.where(spk, d, w)
        for name, arr in (("s", s), ("v", v)):
            full = (
                arr.reshape(B, NSH // F, t_total, F)
                .transpose(0, 1, 3, 2)
                .reshape(B, NSH, t_total)
            )
            if name == "s":
                s_full[:, lo:hi, :] = full
            else:
                v_full[:, lo:hi, :] = full
    return s_full, v_full


def _host_fallback(u, theta_base):
    """Exact numpy simulation; only used if theta_base is non-uniform."""
    u = np.asarray(u, np.float32)
    b, n, t = u.shape
    tb = np.asarray(theta_base, np.float32)[0, :, 0]
    v = np.zeros((b, n), np.float32)
    theta = np.broadcast_to(tb, (b, n)).astype(np.float32).copy()
    ref = np.zeros((b, n), np.float32)
    c = (tb * np.float32(1.0 - BETA)).astype(np.float32)
    ss = np.empty((b, n, t), np.float32)
    vs = np.empty((b, n, t), np.float32)
    for i in range(t):
        u_eff = np.where(ref > 0, np.float32(0.0), u[:, :, i])
        v = (np.float32(ALPHA) * v + u_eff).astype(np.float32)
        s = (v >= theta).astype(np.float32)
        v = (v - s * theta).astype(np.float32)
        ref = np.where(s > 0, np.float32(2.0), np.maximum(ref - 1.0, 0.0).astype(np.float32))
        theta = ((theta * np.float32(BETA) + c) + np.float32(GAMMA) * s).astype(np.float32)
        ss[:, :, i] = s
        vs[:, :, i] = v
    return ss, vs


def run(u, theta_base, t_total=T, tc=TC, trace=False):
    from concourse.bass_utils import run_bass_kernel_spmd

    tb = np.asarray(theta_base, dtype=np.float32)
    nc = _get_nc(t_total, tc, float(tb.flat[0]))
    in_maps = _shard_inputs(u, t_total)
    res = run_bass_kernel_spmd(nc, in_maps, core_ids=list(range(NCORES)), trace=trace)
    s_full, v_full = _unshard(res.results, t_total)
    return (s_full, v_full), res


def kernel(u, theta_base):
    tb = np.asarray(theta_base, dtype=np.float32)
    if not np.all(tb == tb.flat[0]):
        return _host_fallback(u, theta_base)
    (s_full, v_full), _ = run(u, theta_base)
    return s_full, v_full


# revision 4
# speedup vs baseline: 1.0039x; 1.0029x over previous
"""LIF bank Trainium2 Bass kernel, v6.

Per-lane recurrence over T=1000 steps, data-parallel over B*N lanes:
8 cores x 4096 lanes ([128 partitions, 32 free] tiles), time-major chunk
layout [P, T, F].

Structure (vs v4 baseline):
- A-form recurrence removes the post-reset V from the critical cycle:
    A_t   = fl(alpha*W_t + P_{t+1})         (STT)
    WR_t  = sel(D_t >= 0, Fw_t, 0)          (custom mask)
    W_t+1 = A_t - WR_t                      (TT sub)
    D_t+1 = W_t+1 - TH_t                    (TT sub)   [exact spike margin]
    TH_t  = (TH_t-1 * beta + c) + gamma*[D_t >= 0]     (custom affsel)
    P_t+2 = sel(D_t >= 0, 0, u_t+2)         (custom gate; 2-step refractory
                                             = two chained gates)
    Fw_t+1= fl(fl(alpha*TH_t) + P_{t+2})    (STT)
  Cycle = D->MASK->SUBW->DTT (3 hops) instead of W->S->M->W with V inline.
- Device outputs are the raw W and D histories; the host derives
    s = (D >= 0),  v = where(D >= 0, D, W)
  which are exactly the device's own spike decisions / soft-reset values.
- Time-major chunk layout (inner stride 1) avoids strided-write stalls.

Numerics vs the jax reference: everything fp32-exact except the spiking
branch of W (fl(aW+P)-fl(fl(aTH)+P) vs fl(a*fl(W-TH)), ~2ulp), same class
of deviation the v4 baseline already had via its c_imm constant.
"""

import numpy as np

ALPHA = 0.95
BETA = 0.995   # THETA_DECAY
GAMMA = 0.35   # THETA_INC

B, N, T = 16, 2048, 1000
NCORES = 8
NSH = N // NCORES          # 256 neurons per core
P, F = 128, 32             # lanes per core = P*F = B*NSH = 4096
TC = 100                   # timesteps per chunk

_CACHE = {}


def _register_custom_ops():
    import concourse.dve_ops as dvo
    from concourse.dve_spec import (
        Spec, Src0, Src1, C0, C1, C2, Zero, One, select, lower, _has_src1,
        PageIdx,
    )
    from concourse.dve_uop import DveOpSpec

    if "LIF2_MASKGATE" in dvo._SUB_OPCODE_FOR_NAME:
        return {o.name: o for o in dvo.OPS if o.name.startswith("LIF2_")}

    def _mg_ref(in0, in1, s0, s1, imm2):
        pg = np.arange(in0.shape[1], dtype=np.float32).reshape(1, -1, 1)
        return np.where(in0 >= 0, in1 * (1.0 - pg), in1 * pg).astype(np.float32)

    _pg = PageIdx(Zero, One)
    specs = {
        # page 0: sel(D>=0, Src1, 0) = WR;  page 1: sel(D>=0, 0, Src1) = P
        "LIF2_MASKGATE": Spec(
            body=select(Src0 >= Zero, Src1 * (One - _pg), Src1 * _pg),
            reference=_mg_ref,
        ),
        # WR = sel(D >= 0, Fw, 0)
        "LIF2_MASK": Spec(
            body=select(Src0 >= Zero, Src1, Zero),
            reference=lambda in0, in1, s0, s1, imm2: np.where(
                in0 >= 0, in1, np.float32(0.0)).astype(np.float32),
        ),
        # P = sel(D >= 0, 0, u)
        "LIF2_GATE": Spec(
            body=select(Src0 >= Zero, Zero, Src1),
            reference=lambda in0, in1, s0, s1, imm2: np.where(
                in0 >= 0, np.float32(0.0), in1).astype(np.float32),
        ),
        # TH' = (TH*C0 + C1) + (D >= 0)*C2
        "LIF2_AFFSEL": Spec(
            body=(Src0 * C0 + C1) + (Src1 >= Zero) * C2,
            reference=lambda in0, in1, s0, s1, imm2: (
                (in0 * np.float32(s0) + np.float32(s1))
                + (in1 >= 0).astype(np.float32) * np.float32(imm2)
            ).astype(np.float32),
        ),
    }
    new_ops = []
    base = len(dvo.OPS)
    for i, (name, spec) in enumerate(specs.items()):
        opcode = dvo._CUSTOM_DVE_ROW_BASE + base + i
        shas = {}
        for ver in ("v3", "v4"):
            uops = lower(spec, ver=ver)
            shas[ver] = DveOpSpec(
                name=name, opcode=opcode, uops=uops, rd1_en=_has_src1(spec)
            ).sha(ver)
        dvo._SUB_OPCODE_FOR_NAME[name] = opcode
        new_ops.append(dvo.DveOp(name, spec, subdim=(name == "LIF2_MASKGATE"),
                                 uops_sha=shas))
    dvo.OPS.extend(new_ops)
    dvo.CUSTOM_DVE_SPECS.update({o.name: o.spec for o in new_ops})
    return {o.name: o for o in new_ops}


def _build_nc(t_total, tc, tb_val):
    import concourse.bacc as bacc
    import concourse.mybir as mybir
    import concourse.tile as tile

    ops = _register_custom_ops()
    MK, GT, AS = ops["LIF2_MASK"], ops["LIF2_GATE"], ops["LIF2_AFFSEL"]
    MG = ops["LIF2_MASKGATE"]

    f32 = mybir.dt.float32
    op = mybir.AluOpType

    tb = np.float32(tb_val)
    c_imm = float(np.float32(tb * np.float32(1.0 - BETA)))
    atb = float(np.float32(np.float32(ALPHA) * tb))  # fl(alpha*tb)

    nc = bacc.Bacc("TRN2", target_bir_lowering=False, num_devices=NCORES)
    u_d = nc.dram_tensor("u", [P, t_total, F], f32, kind="ExternalInput")
    s_d = nc.dram_tensor("s", [P, t_total, F], f32, kind="ExternalOutput")  # D history
    v_d = nc.dram_tensor("v", [P, t_total, F], f32, kind="ExternalOutput")  # W history

    nchunks = t_total // tc
    assert nchunks * tc == t_total
    vec = nc.vector

    with tile.TileContext(nc) as tc_ctx:
        with (
            tc_ctx.tile_pool(name="state", bufs=1) as st,
            tc_ctx.tile_pool(name="uc", bufs=3) as upool,
            tc_ctx.tile_pool(name="wd", bufs=2) as wpool,
        ):
            th = [st.tile([P, F], f32, tag=f"th{i}", name=f"th{i}") for i in range(2)]
            aa = [st.tile([P, F], f32, tag=f"aa{i}", name=f"aa{i}") for i in range(2)]
            wp = [st.tile([P, 2, F], f32, tag=f"wp{i}", name=f"wp{i}") for i in range(2)]

            uc, wd = {}, {}
            UH = 8  # head slots of chunk 0 loaded separately for fast start

            def load_u(k):
                if k < nchunks and k not in uc:
                    uc[k] = upool.tile([P, tc + 4, F], f32, tag="uc", name=f"uc{k}")
                    hi = min((k + 1) * tc + 2, t_total)
                    nc.sync.dma_start(uc[k][:, 2:2 + hi - k * tc, :],
                                      u_d[:, k * tc:hi, :])

            uh = st.tile([P, UH, F], f32, tag="uh", name="uh")
            nc.sync.dma_start(uh[:], u_d[:, 0:UH, :])

            def u_at(k, g):
                g = min(g, t_total - 1)
                if g < UH:
                    return uh[:, g, :]
                return uc[k][:, 2 + g - k * tc, :]

            load_u(0)
            load_u(1)

            wd[0] = wpool.tile([P, 2, tc + 1, F], f32, tag="wd", name="wd0")

            # ---- prologue: W_0 = u_0, D_0 = W_0 - tb, TH_-1 = tb,
            #      Fw_0 = fl(atb + u_1), A_0 = fl(alpha*W_0 + u_1)
            vec.tensor_copy(wd[0][:, 0, 0, :], uh[:, 0, :])
            vec.tensor_scalar(out=wd[0][:, 1, 0, :], in0=wd[0][:, 0, 0, :],
                              scalar1=float(tb), scalar2=None, op0=op.subtract)
            vec.memset(th[0][:], float(tb))
            vec.tensor_scalar(out=uc[0][:, 0, :], in0=uh[:, 1, :],
                              scalar1=atb, scalar2=None, op0=op.add)
            vec.scalar_tensor_tensor(out=aa[0][:], in0=wd[0][:, 0, 0, :],
                                     scalar=ALPHA, in1=uh[:, 1, :],
                                     op0=op.mult, op1=op.add)

            for t in range(t_total - 1):
                k, tau = t // tc, t % tc
                q, q2 = t % 2, (t + 1) % 2
                if tau == 0:
                    if k > 0:
                        wd[k] = wpool.tile([P, 2, tc + 1, F], f32, tag="wd",
                                           name=f"wd{k}")
                    load_u(k + 2)

                # chunk k's slot 0 lives in wd[k-1] slot tc (no carry copy)
                if tau == 0 and k > 0:
                    d_t = wd[k - 1][:, 1, tc, :]
                else:
                    d_t = wd[k][:, 1, tau, :]
                g = min(t + 2, t_total - 1)
                tg = 2 + g - k * tc
                if g >= UH:
                    # 1+3 merged: [WR_t | P_{t+2}] via paged select
                    #   Src1 pages = [fw-slot q | u-slot tg] of uc[k]
                    step = tg - q
                    vec._custom_dve(
                        MG, out=wp[q][:],
                        in0=d_t.unsqueeze(1).broadcast_to([P, 2, F]),
                        in1=uc[k][:, q:tg + 1:step, :])
                else:
                    vec._custom_dve(MK, out=wp[q][:, 0, :], in0=d_t,
                                    in1=uc[k][:, q, :])
                    vec._custom_dve(GT, out=wp[q][:, 1, :], in0=d_t,
                                    in1=u_at(k, t + 2))
                # 2. TH_t = (TH_{t-1}*beta + c) + gamma*[D_t >= 0]
                vec._custom_dve(AS, out=th[q2][:], in0=th[q][:], in1=d_t,
                                s0=BETA, s1=c_imm, imm2=GAMMA)
                # 4. W_{t+1} = A_t - WR_t
                vec.tensor_tensor(out=wd[k][:, 0, tau + 1, :], in0=aa[q][:],
                                  in1=wp[q][:, 0, :], op=op.subtract)
                # 5. Fw_{t+1} = fl(fl(alpha*TH_t) + P_{t+2}) -> fw slot of chunk(t+1)
                k1 = (t + 1) // tc
                vec.scalar_tensor_tensor(out=uc[k1][:, q2, :], in0=th[q2][:],
                                         scalar=ALPHA, in1=wp[q][:, 1, :],
                                         op0=op.mult, op1=op.add)
                # 6. D_{t+1} = W_{t+1} - TH_t
                vec.tensor_tensor(out=wd[k][:, 1, tau + 1, :],
                                  in0=wd[k][:, 0, tau + 1, :], in1=th[q2][:],
                                  op=op.subtract)
                # 7. A_{t+1} = fl(alpha*W_{t+1} + P_{t+2})
                vec.scalar_tensor_tensor(out=aa[q2][:], in0=wd[k][:, 0, tau + 1, :],
                                         scalar=ALPHA, in1=wp[q][:, 1, :],
                                         op0=op.mult, op1=op.add)

                # stream the chunk's finished slots out in pieces so the
                # final chunk's DMA overlaps compute (slot tau+1 is written
                # by iter tau; slots [lo, tau+1) are final after iter tau)
                last = (k == nchunks - 1)
                cuts = (24, 46, 64, 78, 88, 94, tc - 1) if last else (tc - 10 - 1, tc - 1)
                if tau in cuts or t == t_total - 2:
                    lo = 0 if k == 0 else 1
                    prev_cut = [c for c in cuts if c < tau]
                    if prev_cut:
                        lo = prev_cut[-1] + 2
                    hi = min(tau + 2, t_total - k * tc)
                    if tau == tc - 1 or t == t_total - 2:
                        hi = min(tc + 1, t_total - k * tc)
                    if hi > lo:
                        nc.sync.dma_start(v_d[:, k * tc + lo:k * tc + hi, :],
                                          wd[k][:, 0, lo:hi, :])
                        nc.sync.dma_start(s_d[:, k * tc + lo:k * tc + hi, :],
                                          wd[k][:, 1, lo:hi, :])

    nc.compile()
    return nc


def _get_nc(t_total, tc, tb_val):
    key = (t_total, tc, float(tb_val))
    if key not in _CACHE:
        _CACHE[key] = _build_nc(t_total, tc, tb_val)
    return _CACHE[key]


def _shard_inputs(u, t_total):
    u = np.asarray(u, dtype=np.float32)
    in_maps = []
    for c in range(NCORES):
        lo, hi = c * NSH, (c + 1) * NSH
        # [B, NSH, T] -> [B, NSH//F, F, T] -> [B, NSH//F, T, F] -> [P, T, F]
        uc = np.ascontiguousarray(
            u[:, lo:hi, :t_total]
            .reshape(B, NSH // F, F, t_total)
            .transpose(0, 1, 3, 2)
            .reshape(P, t_total, F)
        )
        in_maps.append({"u": uc})
    return in_maps


def _unshard(res, t_total):
    s_full = np.empty((B, N, t_total), dtype=np.float32)
    v_full = np.empty((B, N, t_total), dtype=np.float32)
    for c in range(NCORES):
        lo, hi = c * NSH, (c + 1) * NSH
        d = res[c]["s"]  # [P, T, F] = D history
        w = res[c]["v"]  # [P, T, F] = W history
        spk = (d >= 0)
        s = spk.astype(np.float32)
        v = np.where(spk, d, w)
        for name, arr in (("s", s), ("v", v)):
            full = (
                arr.reshape(B, NSH // F, t_total, F)
                .transpose(0, 1, 3, 2)
                .reshape(B, NSH, t_total)
            )
            if name == "s":
                s_full[:, lo:hi, :] = full
            else:
                v_full[:, lo:hi, :] = full
    return s_full, v_full


def _host_fallback(u, theta_base):
    """Exact numpy simulation; only used if theta_base is non-uniform."""
    u = np.asarray(u, np.float32)
    b, n, t = u.shape
    tb = np.asarray(theta_base, np.float32)[0, :, 0]
    v = np.zeros((b, n), np.float32)
    theta = np.broadcast_to(tb, (b, n)).astype(np.float32).copy()
    ref = np.zeros((b, n), np.float32)
    c = (tb * np.float32(1.0 - BETA)).astype(np.float32)
    ss = np.empty((b, n, t), np.float32)
    vs = np.empty((b, n, t), np.float32)
    for i in range(t):
        u_eff = np.where(ref > 0, np.float32(0.0), u[:, :, i])
        v = (np.float32(ALPHA) * v + u_eff).astype(np.float32)
        s = (v >= theta).astype(np.float32)
        v = (v - s * theta).astype(np.float32)
        ref = np.where(s > 0, np.float32(2.0), np.maximum(ref - 1.0, 0.0).astype(np.float32))
        theta = ((theta * np.float32(BETA) + c) + np.float32(GAMMA) * s).astype(np.float32)
        ss[:, :, i] = s
        vs[:, :, i] = v
    return ss, vs


def run(u, theta_base, t_total=T, tc=TC, trace=False):
    from concourse.bass_utils import run_bass_kernel_spmd

    tb = np.asarray(theta_base, dtype=np.float32)
    nc = _get_nc(t_total, tc, float(tb.flat[0]))
    in_maps = _shard_inputs(u, t_total)
    res = run_bass_kernel_spmd(nc, in_maps, core_ids=list(range(NCORES)), trace=trace)
    s_full, v_full = _unshard(res.results, t_total)
    return (s_full, v_full), res


def kernel(u, theta_base):
    tb = np.asarray(theta_base, dtype=np.float32)
    if not np.all(tb == tb.flat[0]):
        return _host_fallback(u, theta_base)
    (s_full, v_full), _ = run(u, theta_base)
    return s_full, v_full
